# revision 13
# baseline (speedup 1.0000x reference)
"""Multi-head attention (B=2, S=2048, d_model=1024, H=16, dk=64) on 8 trn2 cores.

Sharding: batch*heads. Core c owns batch b=c//4 and heads 4*(c%4)..4*(c%4)+3
(channel rows 256*(c%4)..+256). Weights are sliced per core on the host; the
per-batch activations/mask are replicated across the 4 cores of that batch.

Device-side layout (all per-core, bf16 unless noted):
  xq/xk/xv : [128, 8*2048]  x^T packed d_model-chunk-major (chunk k at cols
             [2048k, 2048k+2048)), x^T[i, s] = x[s, i] for this core's batch.
  wq/wk/wv : [128, 8*256]   W^T slice packed the same way ([1024, 256] ->
             chunk k at cols [256k, 256k+256)).  wq is pre-scaled by 1/8
             (the 1/sqrt(dk) of the reference, folded into Q).
  maskT    : [128, 16*2048] mask[b]^T (keys on partitions): chunk kt at cols
             [2048kt, +2048) holds key rows [128kt, 128kt+128) x all queries.
  outT     : [256, 2048] f32 output, outT[j, s] = out[b, s, 256*(c%4)+j].

Math per head (transposed-score orientation, keys on partitions):
  qT = (Wq/8) @ x^T, kT = Wk @ x^T     (PE, bf16, PSUM f32)
  v  = x^T-chunks^T @ Wv^T             (natural [S, dk] layout, augmented with
                                        a ones column -> denominator for free)
  scT[j, i] = sum_d kT[d, j] qT[d, i]  (K=64; two heads packed in the array
                                        via partition offsets 0/64)
  P = exp(scT)            (ACT, PSUM->SBUF bf16)
  P = P * maskT           (DVE, bf16 2x)  [multiplicative mask == where(m==0,-1e9)]
  av = sum_kt V'_kt^T P_kt (PE, accumulated in PSUM; row 64 = sum_j P = denom)
  outT = av[0:64] * (1/av[64])  (DVE reciprocal + partition-broadcast multiply)
"""

import os
from contextlib import ExitStack

import ml_dtypes
import numpy as np

import concourse.bass as bass
import concourse.mybir as mybir
import concourse.tile as tile
from concourse import bacc
from concourse.bass_utils import run_bass_kernel_spmd

BF16 = mybir.dt.bfloat16
F32 = mybir.dt.float32
NPBF16 = np.dtype(ml_dtypes.bfloat16)

B, S, DM, H, DK = 2, 2048, 1024, 16, 64
NCORES = 8
P = 128
NKC = DM // P        # 8 d_model chunks
KT = S // P          # 16 key tiles
HPC = 4              # heads per core
CW = HPC * DK        # 256 channel rows per core

LAST_RESULTS = None  # BassKernelResults of the most recent run (for test.py)

_NC = None


def _build_bass():
    nc = bacc.Bacc(trn_type="TRN2", debug=False)

    xq_d = nc.dram_tensor("xq", [P, NKC * S], BF16, kind="ExternalInput").ap()
    xk_d = nc.dram_tensor("xk", [P, NKC * S], BF16, kind="ExternalInput").ap()
    xv_d = nc.dram_tensor("xv", [P, NKC * S], BF16, kind="ExternalInput").ap()
    wq_d = nc.dram_tensor("wq", [P, NKC * CW], BF16, kind="ExternalInput").ap()
    wk_d = nc.dram_tensor("wk", [P, NKC * CW], BF16, kind="ExternalInput").ap()
    wv_d = nc.dram_tensor("wv", [P, NKC * CW], BF16, kind="ExternalInput").ap()
    mask_d = nc.dram_tensor("maskT", [P, KT * S], BF16, kind="ExternalInput").ap()
    out_d = nc.dram_tensor("outT", [CW, S], F32, kind="ExternalOutput").ap()

    with tile.TileContext(nc) as tc:
        with ExitStack() as ctx:
            _body(ctx, tc, nc, xq_d, xk_d, xv_d, wq_d, wk_d, wv_d, mask_d, out_d)
    nc.compile()  # bacc passes: split multi-wait instructions (TRN2 allows 1)
    return nc


def _body(ctx, tc, nc, xq_d, xk_d, xv_d, wq_d, wk_d, wv_d, mask_d, out_d):
    EXP = mybir.ActivationFunctionType.Exp
    MULT = mybir.AluOpType.mult

    persist = ctx.enter_context(tc.tile_pool(name="persist", bufs=1))
    xpool = ctx.enter_context(tc.tile_pool(name="x", bufs=3))
    ppool = ctx.enter_context(tc.tile_pool(name="p", bufs=3))
    recpool = ctx.enter_context(tc.tile_pool(name="rec", bufs=2))
    outpool = ctx.enter_context(tc.tile_pool(name="ot", bufs=3))

    # --- resident tiles -----------------------------------------------------
    wq_sb = persist.tile([P, NKC * CW], BF16, tag="wq", name="wq_sb")
    wk_sb = persist.tile([P, NKC * CW], BF16, tag="wk", name="wk_sb")
    wv_sb = persist.tile([P, NKC * CW], BF16, tag="wv", name="wv_sb")
    nc.sync.dma_start(wq_sb[:], wq_d[:, :])
    nc.sync.dma_start(wk_sb[:], wk_d[:, :])
    nc.sync.dma_start(wv_sb[:], wv_d[:, :])

    # qT/kT: tile `pair` holds heads (2*pair, 2*pair+1) on partitions 0:64/64:128
    qT = [persist.tile([P, S], BF16, tag=f"qT{m}", name=f"qT{m}") for m in range(2)]
    kT = [persist.tile([P, S], BF16, tag=f"kT{m}", name=f"kT{m}") for m in range(2)]
    # v in natural [S, dk] layout, per (key-tile, head) blocks of 65 columns
    # (64 value columns + a ones column feeding the softmax denominator).
    vaug = persist.tile([P, KT * HPC * 65], BF16, tag="vaug", name="vaug")
    nc.gpsimd.memset(vaug[:], 1.0)

    mask_sb = [persist.tile([P, S], BF16, tag=f"mask{t}", name=f"mask{t}") for t in range(KT)]

    # --- projections (own PSUM pool scope; PSUM pools are stack-allocated) ---
    proj_psum = ExitStack()
    ppsum = proj_psum.enter_context(tc.tile_pool(name="ppsum", bufs=8, space="PSUM"))

    def qk_proj(x_d, w_sb, dstT, copy_engine):
        ps = [ppsum.tile([P, 512], F32, tag="pp", name="pp") for _ in range(8)]
        for k in range(NKC):
            xt = xpool.tile([P, S], BF16, tag="x", name="xt")
            # SWDGE: slot reuse gives these loads many WAR waits, which the
            # HWDGE DIRECT2D codegen path rejects ("too many sync waits")
            nc.gpsimd.dma_start(xt[:], x_d[:, k * S : (k + 1) * S])
            for m in range(2):
                for n in range(4):
                    nc.tensor.matmul(
                        ps[m * 4 + n][:],
                        w_sb[:, k * CW + m * P : k * CW + (m + 1) * P],
                        xt[:, n * 512 : (n + 1) * 512],
                        start=(k == 0),
                        stop=(k == NKC - 1),
                    )
        for m in range(2):
            for n in range(4):
                copy_engine(out=dstT[m][:, n * 512 : (n + 1) * 512], in_=ps[m * 4 + n][:])

    def scalar_copy(out, in_):
        nc.scalar.copy(out=out, in_=in_)

    def vector_copy(out, in_):
        nc.vector.tensor_copy(out, in_)

    qk_proj(xq_d, wq_sb, qT, scalar_copy)
    qk_proj(xk_d, wk_sb, kT, vector_copy)

    proj_psum.close()

    # v projection: out v[s, j] -- lhsT = x^T chunk slices, moving = W^T chunks.
    # m-outer so each [128, 256] output accumulates its 8 k-steps back-to-back
    # into one full-bank psum tile (interleaved groups can't share a bank).
    v_psum = ExitStack()
    vpsum = v_psum.enter_context(tc.tile_pool(name="vpsum", bufs=4, space="PSUM"))
    xvt = [xpool.tile([P, S], BF16, tag=f"xv{k}", name=f"xv{k}", bufs=1) for k in range(NKC)]
    for k in range(NKC):
        nc.gpsimd.dma_start(xvt[k][:], xv_d[:, k * S : (k + 1) * S])
    for m in range(KT):
        vp = vpsum.tile([P, CW], F32, tag="vp", name="vp")
        for k in range(NKC):
            nc.tensor.matmul(
                vp[:],
                xvt[k][:, m * P : (m + 1) * P],
                wv_sb[:, k * CW : (k + 1) * CW],
                start=(k == 0),
                stop=(k == NKC - 1),
            )
        src = vp[:].rearrange("p (h x) -> p h x", x=DK)
        dst = vaug[:, m * (HPC * 65) : (m + 1) * (HPC * 65)].rearrange(
            "p (h x) -> p h x", x=65
        )[:, :, 0:DK]
        if m % 2 == 0:
            nc.scalar.copy(out=dst, in_=src)
        else:
            nc.vector.tensor_copy(dst, src)
    v_psum.close()

    # mask loads (issued after x/w so they don't delay the projections)
    for t in range(KT):
        nc.sync.dma_start(mask_sb[t][:], mask_d[:, t * S : (t + 1) * S])

    # --- attention ----------------------------------------------------------
    # NB: a plain Internal DRAM tensor makes LoadExecutable fail under the
    # axon PJRT path, so the reciprocal bounce buffer is a (ignored) output.
    rec_dram = nc.dram_tensor("rec_bounce", [8, 1024], F32, kind="ExternalOutput").ap()
    scpool = ctx.enter_context(tc.tile_pool(name="scps", bufs=1, space="PSUM"))
    avpool = ctx.enter_context(tc.tile_pool(name="avps", bufs=2, space="PSUM"))
    for qh in range(2):        # query half: columns [qh*1024, +1024)
        for pair in range(2):  # heads (2*pair, 2*pair+1)
            av = [avpool.tile([65, 1024], F32, tag="av", name="av") for _ in range(2)]
            for kt in range(KT):
                sc = scpool.tile([P, 2048], F32, tag="sc", name="sc")
                for h01 in range(2):
                    pb = h01 * 64
                    for nn in range(2):
                        nc.tensor.matmul(
                            sc[:, h01 * 1024 + nn * 512 : h01 * 1024 + (nn + 1) * 512],
                            kT[pair][pb : pb + 64, kt * P : (kt + 1) * P],
                            qT[pair][pb : pb + 64, qh * 1024 + nn * 512 : qh * 1024 + (nn + 1) * 512],
                            start=True,
                            stop=True,
                        )
                p = ppool.tile([P, 2048], BF16, tag="p", name="pt")
                nc.scalar.activation(out=p[:], in_=sc[:], func=EXP)
                for h01 in range(2):
                    nc.vector.tensor_tensor(
                        out=p[:, h01 * 1024 : (h01 + 1) * 1024],
                        in0=p[:, h01 * 1024 : (h01 + 1) * 1024],
                        in1=mask_sb[kt][:, qh * 1024 : qh * 1024 + 1024],
                        op=MULT,
                    )
                for h01 in range(2):
                    head = pair * 2 + h01
                    blk = (kt * HPC + head) * 65
                    for nn in range(2):
                        nc.tensor.matmul(
                            av[h01][:, nn * 512 : (nn + 1) * 512],
                            vaug[:, blk : blk + 65],
                            p[:, h01 * 1024 + nn * 512 : h01 * 1024 + (nn + 1) * 512],
                            start=(kt == 0),
                            stop=(kt == KT - 1),
                        )
            for h01 in range(2):
                head = pair * 2 + h01
                rec = recpool.tile([1, 1024], F32, tag="rec", name="rec")
                nc.vector.reciprocal(rec[:], av[h01][64:65, :])
                # engines can't read 0-stride partition APs; bounce the recip
                # row through DRAM and broadcast-read it across 64 partitions
                idx = (qh * 2 + pair) * 2 + h01
                nc.sync.dma_start(out=rec_dram[idx : idx + 1, :], in_=rec[0:1, :])
                rec_b = recpool.tile([64, 1024], F32, tag="rec_b", name="rec_b")
                nc.gpsimd.dma_start(
                    out=rec_b[:], in_=rec_dram[idx, :].partition_broadcast(64)
                )
                ot = outpool.tile([64, 1024], F32, tag="ot", name="ot")
                nc.vector.tensor_tensor(
                    out=ot[:],
                    in0=av[h01][0:64, :],
                    in1=rec_b[:],
                    op=MULT,
                )
                nc.sync.dma_start(
                    out_d[head * 64 : (head + 1) * 64, qh * 1024 : (qh + 1) * 1024],
                    ot[:],
                )


def get_nc():
    global _NC
    if _NC is None:
        _NC = _build_bass()
    return _NC


# ---------------------------------------------------------------------------
# host-side packing
# ---------------------------------------------------------------------------

def _pack_x(x):
    """[S, DM] f32 -> x^T chunk-packed [128, NKC*S] bf16."""
    xt = np.ascontiguousarray(np.asarray(x, np.float32).T)  # [DM, S]
    return np.ascontiguousarray(
        xt.reshape(NKC, P, S).transpose(1, 0, 2).reshape(P, NKC * S)
    ).astype(NPBF16)


def _pack_w(w, rows0, scale=1.0):
    """W[1024,1024] -> W^T slice chunk-packed [128, NKC*CW] bf16."""
    wt = np.ascontiguousarray((np.asarray(w, np.float32)[rows0 : rows0 + CW, :] * scale).T)
    return np.ascontiguousarray(
        wt.reshape(NKC, P, CW).transpose(1, 0, 2).reshape(P, NKC * CW)
    ).astype(NPBF16)


def _pack_mask(m):
    """mask[b] [S, S] int -> mask^T chunk-packed [128, KT*S] bf16 (0/1)."""
    mt = np.ascontiguousarray(np.asarray(m).T.astype(np.float32))  # [S_keys, S_q]
    return np.ascontiguousarray(
        mt.reshape(KT, P, S).transpose(1, 0, 2).reshape(P, KT * S)
    ).astype(NPBF16)


def kernel(query, key, value, mask, Wq, bq, Wk, bk, Wv, bv):
    global LAST_RESULTS
    nc = get_nc()

    query = np.asarray(query, np.float32)
    key = np.asarray(key, np.float32)
    value = np.asarray(value, np.float32)
    mask = np.asarray(mask)

    xq = [_pack_x(query[b]) for b in range(B)]
    xk = [_pack_x(key[b]) for b in range(B)]
    xv = [_pack_x(value[b]) for b in range(B)]
    mk = [_pack_mask(mask[b]) for b in range(B)]

    in_maps = []
    for c in range(NCORES):
        b = c // 4
        rows0 = (c % 4) * CW
        in_maps.append(
            {
                "xq": xq[b],
                "xk": xk[b],
                "xv": xv[b],
                "wq": _pack_w(Wq, rows0, scale=1.0 / np.sqrt(DK)),
                "wk": _pack_w(Wk, rows0),
                "wv": _pack_w(Wv, rows0),
                "maskT": mk[b],
            }
        )

    LAST_RESULTS = run_bass_kernel_spmd(
        nc,
        in_maps,
        core_ids=list(range(NCORES)),
        trace=bool(os.environ.get("BASS_TRACE")),
    )

    out = np.empty((B, S, DM), np.float32)
    for c in range(NCORES):
        b = c // 4
        rows0 = (c % 4) * CW
        out[b, :, rows0 : rows0 + CW] = LAST_RESULTS.results[c]["outT"].T
    return out
